# revision 23
# baseline (speedup 1.0000x reference)
"""AttentionalCopula Trainium2 kernel, v2.

Data-parallel over batch: 8 NeuronCores, 2 batch elements per core,
both elements merged into one instruction stream (weights shared, free
dim N=512).  All matmuls in bf16 (fp32 PSUM accumulate); vector math in
fp32 where it matters (LN stats, softmax denominators, loss head).
Biases folded into ACT-engine PSUM evacuations (per-partition bias),
u-row handled as a K=2 matmul pass.  Fully-masked score block skipped.
Single ACT table set (exp/ln/identity/relu) to avoid table reloads.

Self-contained: hardcodes shapes from the problem spec.
"""
import math
import sys

import numpy as np
import ml_dtypes

sys.path.insert(0, "/opt/trn_rl_repo")

import concourse.bass as bass  # noqa: E402
import concourse.bacc as bacc  # noqa: E402
import concourse.tile as tile  # noqa: E402
import concourse.mybir as mybir  # noqa: E402
from contextlib import ExitStack  # noqa: E402

F32 = mybir.dt.float32
BF16 = mybir.dt.bfloat16
AF = mybir.ActivationFunctionType
ALU = mybir.AluOpType

B, D, NH, NS, NT = 16, 256, 512, 8, 32
NV = NS * NT          # 256
L, H, A = 4, 8, 64
HA = H * A            # 512
M = 512
R = 128
W = NH + NV           # 768
EPS = 1e-5
SCALE = A ** -0.5
NCORES = 8
EPC = B // NCORES     # 2 elems per core
W2 = 2 * W            # 1536 (both elems, e-major)
NV2 = 2 * NV          # 512

BF = ml_dtypes.bfloat16

_BUILD_CACHE = {}


def ts(i, n):
    return slice(i * n, (i + 1) * n)


def _build(ln_affine, kv_bias, fast_ln1, fast_head):
    nc = bacc.Bacc(None, target_bir_lowering=False)

    def P(name, shape, out=False, dt=BF16):
        return nc.declare_dram_parameter(name, shape, dt, isOutput=out)

    # single-DMA blobs: SP-queue DMA issue costs ~1us each, so pack
    # aggressively.  bf16 const blob layout (cols):
    #   ident 0:128 | maskm 128:256 | dsw 256:1280 | dew 1280:1792
    #   | ubc 1792:3328
    # f32 const blob layout (cols):
    #   dsb 0:4 | ucol 4:16 | ffb 16:64 | onehot 64:576 | onescol 576
    #   | wv0 577 | kwu 578:594 | kbc 594:610 (kv_bias only)
    # per-layer weight blob (bf16):
    #   kw 0:1024 | vw 1024:2048 | vwu 2048:2560 | f1 2560:4608
    #   | f2 4608:6656 | f3 6656:8704 | vbb 8704:9216 (kv_bias only)
    CBN = 3328
    CFN = 610 if kv_bias else 594
    WBN = 9216 if kv_bias else 8704
    cb_d = P("cb", (128, CBN))
    cf_d = P("cf", (128, CFN), dt=F32)
    wb_d = P("wb", (L, 128, WBN))
    ki0_d = P("ki0", (128, W2))
    ki1_d = P("ki1", (128, W2))
    sr_d = P("smallrow", (1, 256))
    if ln_affine:
        lnp_d = P("lnp", (1, L * 4 * HA), dt=F32)
    out_d = P("out", (1, EPC), out=True, dt=F32)

    with tile.TileContext(nc) as tc, ExitStack() as ctx:
        const = ctx.enter_context(tc.tile_pool(name="const", bufs=1))
        kpool = ctx.enter_context(tc.tile_pool(name="kvw", bufs=2))
        fpool = ctx.enter_context(tc.tile_pool(name="ffw", bufs=2))
        kvpool = ctx.enter_context(tc.tile_pool(name="keys", bufs=2))
        epool = ctx.enter_context(tc.tile_pool(name="exp", bufs=3))
        apool = ctx.enter_context(tc.tile_pool(name="att", bufs=3))
        rpool = ctx.enter_context(tc.tile_pool(name="attres", bufs=2))
        tpool = ctx.enter_context(tc.tile_pool(name="attT", bufs=3))
        ftpool = ctx.enter_context(tc.tile_pool(name="ffT", bufs=3))
        spool = ctx.enter_context(tc.tile_pool(name="small", bufs=4))
        ps_b = ctx.enter_context(tc.tile_pool(name="ps_b", bufs=2, space="PSUM"))
        ps_sc = ctx.enter_context(tc.tile_pool(name="ps_sc", bufs=2, space="PSUM"))
        ps_a = ctx.enter_context(tc.tile_pool(name="ps_a", bufs=2, space="PSUM"))

        dma = nc.sync.dma_start

        # ---- constants / inputs (few big DMAs) ----
        cb = const.tile([128, CBN], BF16, tag="cb")
        dma(cb[:], cb_d.ap())
        ki0 = const.tile([128, W2], BF16, tag="ki0")
        dma(ki0[:], ki0_d.ap())
        ki1 = const.tile([128, W2], BF16, tag="ki1")
        dma(ki1[:], ki1_d.ap())
        cf = const.tile([128, CFN], F32, tag="cf")
        dma(cf[:], cf_d.ap())
        srow = const.tile([1, 256], BF16, tag="srow")
        dma(srow[:], sr_d.ap())
        ident = cb[:, 0:128]
        maskm = cb[:, 128:256]
        dsw_t = cb[:, 256:1280].rearrange("p (a n) -> p a n", a=2)
        dew_t = cb[:, 1280:1792].rearrange("p (a n) -> p a n", a=4)
        u_bcast = cb[:, 1792:3328]
        dsb_t = cf[:, 0:4]
        u_col = cf[:, 4:16]
        ffb_t = cf[:, 16:64]
        onehot_t = cf[:, 64:576].rearrange("p (a n) -> p a n", a=4)
        ones_col = cf[:, 576:577]
        wv0 = cf[:, 577:578]
        ones_row = srow[0:1, 0:128]
        deb_t = srow[0:1, 128:256]
        if ln_affine:
            lnp_t = const.tile([1, 16, HA], F32, tag="lnp")
            dma(lnp_t[:], lnp_d.ap().rearrange("p (a n) -> p a n", a=16))
        eps_t = const.tile([128, 1], F32, tag="eps")
        nc.gpsimd.memset(eps_t[:], EPS)
        sc8_t = const.tile([128, 1], F32, tag="sc8")
        nc.gpsimd.memset(sc8_t[:], SCALE)
        neg1_t = const.tile([1, 1], F32, tag="neg1")
        nc.gpsimd.memset(neg1_t[:], -1.0)
        fbias_t = const.tile([1, 1], F32, tag="fbias")
        nc.gpsimd.memset(fbias_t[:], -(NV - 1) * math.log(R))
        res_sb = const.tile([1, EPC], F32, tag="res")

        evac_ctr = [0]

        def evac(out_ap, in_ap):
            # PSUM->SBUF copies alternating DVE / ACT
            if evac_ctr[0] % 2 == 0:
                nc.vector.tensor_copy(out_ap, in_ap)
            else:
                nc.scalar.copy(out_ap, in_ap)
            evac_ctr[0] += 1

        def mm(ps_ap, chunks):
            n = len(chunks)
            for i, (lh, rh) in enumerate(chunks):
                nc.tensor.matmul(ps_ap, lh, rh,
                                 start=(i == 0), stop=(i == n - 1))

        def ln_stats(mvall, in_ap, ev):
            """bn stats for one [128, HA] chunk -> mvall[:, ev, 0:2]."""
            st6 = spool.tile([128, 6], F32, tag="st6")
            nc.vector.bn_stats(st6[:], in_ap)
            nc.vector.bn_aggr(mvall[:, ev, :], st6[:])

        def ln_batch(mvall):
            """rstd/-mean*rstd for all 4 chunks in two ACT calls.
            rstd via exp(-0.5*ln(var+eps)) keeps Ln/Exp adjacent so only
            two table loads per LN phase."""
            lv = spool.tile([128, 4], F32, tag="lv")
            nc.scalar.activation(lv[:], mvall[:, :, 1], AF.Ln,
                                 bias=eps_t[:, 0:1])
            rs = spool.tile([128, 4], F32, tag="rs")
            nc.scalar.activation(rs[:], lv[:], AF.Exp, scale=-0.5)
            nb = spool.tile([128, 4], F32, tag="nb")
            nc.vector.scalar_tensor_tensor(nb[:], mvall[:, :, 0], -1.0, rs[:],
                                           op0=ALU.mult, op1=ALU.mult)
            return rs, nb

        def ln_apply(out_ap, in_ap, rs, nb, ev, l, which):
            """normalize on DVE: out = in*rstd - mean*rstd."""
            if not ln_affine:
                nc.vector.tensor_scalar(out_ap, in_ap,
                                        rs[:, ev:ev + 1], nb[:, ev:ev + 1],
                                        op0=ALU.mult, op1=ALU.add)
            else:
                t0 = spool.tile([128, HA], F32, tag="lnt0", bufs=1)
                nc.vector.tensor_scalar(t0[:], in_ap,
                                        rs[:, ev:ev + 1], nb[:, ev:ev + 1],
                                        op0=ALU.mult, op1=ALU.add)
                gb = spool.tile([128, HA], F32, tag="lngb", bufs=1)
                gi = l * 4 + (0 if which == 1 else 2)
                nc.gpsimd.partition_broadcast(gb[:], lnp_t[0:1, gi, :])
                nc.vector.tensor_mul(t0[:], t0[:], gb[:])
                nc.gpsimd.partition_broadcast(gb[:], lnp_t[0:1, gi + 1, :])
                nc.vector.tensor_add(out_ap, t0[:], gb[:])

        # vals denominator column (col 64 = 1.0, written once; col 65 is
        # never read).  GPSIMD memsets instead of a slow strided DMA.
        vals_a = kvpool.tile([128, 12, 8, 66], BF16, tag="vals")
        vals_b = kvpool.tile([128, 12, 8, 66], BF16, tag="vals")
        for vt_ in (vals_a, vals_b):
            for wt in range(12):
                nc.gpsimd.memset(vt_[:, wt, :, 64:65], 1.0)
        vals_bufs = [vals_a, vals_b]

        # ================== initial att / attT ==================
        # attT[ha, (e,v)] = ds_w.T @ pred_encoded.T + ds_b (per-partition bias)
        attT = tpool.tile([128, 4, NV2], BF16, tag="attT")
        for t in range(4):
            ps = ps_b.tile([128, 512], F32, tag="psb")
            for e in range(EPC):
                nc.tensor.matmul(ps[:, ts(e, 256)],
                                 dsw_t[:, 0, ts(t, 128)],
                                 ki0[:, e * W + NH: (e + 1) * W],
                                 start=True, stop=False)
                nc.tensor.matmul(ps[:, ts(e, 256)],
                                 dsw_t[:, 1, ts(t, 128)],
                                 ki1[:, e * W + NH: (e + 1) * W],
                                 start=False, stop=True)
            nc.scalar.activation(attT[:, t, :], ps[:], AF.Identity,
                                 bias=dsb_t[:, t: t + 1])
        # att natural [v, (e,vt,ha)] via PE transposes
        att = apool.tile([128, 4, HA], BF16, tag="att")
        for ev in range(4):
            e, vt = divmod(ev, 2)
            ps = ps_b.tile([128, 512], BF16, tag="psb")
            for t in range(4):
                nc.tensor.transpose(ps[:, ts(t, 128)],
                                    attT[:, t, e * 256 + vt * 128:
                                         e * 256 + (vt + 1) * 128],
                                    ident)
            evac(att[:, ev, :], ps[:])

        # ================== layers ==================
        def load_weights(l):
            wt = kpool.tile([128, WBN], BF16, tag="wb")
            dma(wt[:], wb_d.ap()[l])
            kw_t = wt[:, 0:1024].rearrange("p (a n) -> p a n", a=2)
            vw_t = wt[:, 1024:2048].rearrange("p (a n) -> p a n", a=2)
            vwu_t = wt[:, 2048:2560]
            ffw1_t = wt[:, 2560:4608].rearrange("p (a n) -> p a n", a=4)
            ffw2_t = wt[:, 4608:6656].rearrange("p (a n) -> p a n", a=4)
            ffw3_t = wt[:, 6656:8704].rearrange("p (a n) -> p a n", a=4)
            kwu_t = cf[:, 578 + l * 4: 578 + (l + 1) * 4]
            kbc_t = cf[:, 594 + l * 4: 594 + (l + 1) * 4] if kv_bias else None
            vbb_t = wt[:, 8704:9216] if kv_bias else None
            return (kw_t, vw_t, kwu_t, vwu_t, kbc_t, vbb_t,
                    ffw1_t, ffw2_t, ffw3_t)

        wts = {0: load_weights(0)}
        kv_tiles = {}

        def emit_kv(l, part):
            """KV compute for layer l, split so part 0 can fill the LN2
            bubble of layer l-1 and part 1 runs after att2T transposes."""
            kw_t, vw_t, kwu_t, vwu_t, kbc_t, vbb_t = wts[l][:6]
            if part == 0:
                keysT = kvpool.tile([128, 4, W2], BF16, tag="keys")
                kv_tiles[l] = (keysT, vals_bufs[l % 2])
                rng_v, rng_k = range(0, 6), []
            elif part == 1:
                rng_v, rng_k = range(6, 12), []
            else:
                rng_v = range(0, 0)
                rng_k = [(t, ch) for t in range(4) for ch in range(3)]
            keysT, vals = kv_tiles[l]
            for wt in rng_v:
                ps = ps_b.tile([128, 512], F32, tag="psb")
                mm(ps[:], [(ki0[:, ts(wt, 128)], vw_t[:, 0, :]),
                           (ki1[:, ts(wt, 128)], vw_t[:, 1, :])])
                # u-term folded into the evac: vals += u[w] * vw_u[ha]
                nc.vector.scalar_tensor_tensor(
                    vals[:, wt, :, 0:64],
                    vwu_t.rearrange("p (h a) -> p h a", h=8),
                    u_col[:, wt:wt + 1],
                    ps[:].rearrange("p (h a) -> p h a", h=8),
                    op0=ALU.mult, op1=ALU.add)
                if kv_bias:
                    nc.vector.tensor_add(
                        vals[:, wt, :, 0:64], vals[:, wt, :, 0:64],
                        vbb_t.rearrange("p (h a) -> p h a", h=8))
            for t, ch in rng_k:
                ps = ps_b.tile([128, 512], F32, tag="psb")
                mm(ps[:], [(kw_t[:, 0, ts(t, 128)], ki0[:, ts(ch, 512)]),
                           (kw_t[:, 1, ts(t, 128)], ki1[:, ts(ch, 512)])])
                # u-term folded into the evac: keys += kw_u[ha] * u[w]
                nc.vector.scalar_tensor_tensor(
                    keysT[:, t, ts(ch, 512)],
                    u_bcast[:, ts(ch, 512)], kwu_t[:, t:t + 1], ps[:],
                    op0=ALU.mult, op1=ALU.add)
                if kv_bias:
                    nc.vector.tensor_scalar_add(
                        keysT[:, t, ts(ch, 512)], keysT[:, t, ts(ch, 512)],
                        kbc_t[:, t:t + 1])

        emit_kv(0, 0)
        emit_kv(0, 1)
        emit_kv(0, 2)

        for l in range(L):
            if l + 1 < L:
                wts[l + 1] = load_weights(l + 1)
            keysT, vals = kv_tiles[l]
            ffw1_t, ffw2_t, ffw3_t = wts[l][6], wts[l][7], wts[l][8]

            # ---- attention ----
            att_res = rpool.tile([128, 4, HA], F32, tag="attres")
            mv1 = spool.tile([128, 4, 2], F32, tag="mv")
            att1 = apool.tile([128, 4, HA], BF16, tag="att")
            att1T = tpool.tile([128, 4, NV2], BF16, tag="attT")
            for e in range(EPC):
                for h in range(H):
                    t, base = h // 2, (h % 2) * 64
                    kslc = keysT[base:base + 64, t, :]
                    aslc = attT[base:base + 64, t, :]
                    # scores S^T[w, v] in two psum tiles of 3 w-chunks each
                    psA = ps_sc.tile([128, 768], F32, tag="sc")
                    for wt in range(3):
                        nc.tensor.matmul(
                            psA[:, ts(wt, 256)],
                            kslc[:, e * W + wt * 128: e * W + (wt + 1) * 128],
                            aslc[:, ts(e, 256)], start=True, stop=True)
                    psB = ps_sc.tile([128, 768], F32, tag="sc")
                    for wt in range(3, 5):
                        nc.tensor.matmul(
                            psB[:, ts(wt - 3, 256)],
                            kslc[:, e * W + wt * 128: e * W + (wt + 1) * 128],
                            aslc[:, ts(e, 256)], start=True, stop=True)
                    # wt=5: v-chunk 0 fully masked -> compute v-chunk 1 only
                    nc.tensor.matmul(
                        psB[:, 640:768],
                        kslc[:, e * W + 640: e * W + 768],
                        aslc[:, e * 256 + 128: e * 256 + 256],
                        start=True, stop=True)
                    expT = epool.tile([128, 1536], BF16, tag="exp")
                    nc.scalar.activation(expT[:, 0:768], psA[:], AF.Exp,
                                         scale=sc8_t[:, 0:1])
                    nc.scalar.activation(expT[:, 768:1536], psB[:], AF.Exp,
                                         scale=sc8_t[:, 0:1])
                    # masking on GPSIMD (SBUF-only engine, otherwise idle)
                    nc.gpsimd.memset(expT[:, 1280:1408], 0.0)
                    nc.gpsimd.tensor_mul(expT[:, 1024:1152],
                                         expT[:, 1024:1152], maskm[:])
                    nc.gpsimd.tensor_mul(expT[:, 1408:1536],
                                         expT[:, 1408:1536], maskm[:])
                    # attention-out directly in [v, a] layout:
                    # lhsT = expT v-slice, rhs = vals (with ones col 64
                    # giving the softmax denominator at out col 64)
                    rec = spool.tile([128, 2], F32, tag="rec")
                    for vc in range(2):
                        ps_at = ps_a.tile([128, 66], F32, tag="a")
                        wts_ao = range(5) if vc == 0 else range(6)
                        last = wts_ao[-1]
                        for wt in wts_ao:
                            nc.tensor.matmul(
                                ps_at[:],
                                expT[:, wt * 256 + vc * 128:
                                     wt * 256 + (vc + 1) * 128],
                                vals[:, e * 6 + wt, h, :],
                                start=(wt == 0), stop=(wt == last))
                        nc.vector.reciprocal(rec[:, vc:vc + 1],
                                             ps_at[:, 64:65])
                        nc.vector.scalar_tensor_tensor(
                            att_res[:, e * 2 + vc, ts(h, 64)],
                            ps_at[:, 0:64], rec[:, vc:vc + 1],
                            att[:, e * 2 + vc, ts(h, 64)],
                            op0=ALU.mult, op1=ALU.add)
                # LN1 stats for this element's two halves, overlapped with
                # the other element's attention
                ln_stats(mv1, att_res[:, e * 2, :], e * 2)
                ln_stats(mv1, att_res[:, e * 2 + 1, :], e * 2 + 1)
                if fast_ln1 and e == 0:
                    # e0's mean-subtract runs during e1's attention
                    for ev in (0, 1):
                        nc.vector.tensor_scalar_sub(
                            att1[:, ev, :], att_res[:, ev, :], mv1[:, ev, 0:1])

            # ---- LN1 + att1T ----
            # ev 0/1 were mean-subtracted during e1's attention, so their
            # transposes go FIRST (psum ring + ACT evacs, nothing queued
            # behind the vals evacs on DVE); then next layer's vals groups
            # keep the PE busy while ev 2/3 resolve.
            def att1t_ev(ev, act_evac):
                e, vt = divmod(ev, 2)
                ps_tr = ps_b.tile([128, 512], BF16, tag="psb")
                for c in range(4):
                    nc.tensor.transpose(ps_tr[:, ts(c, 128)],
                                        att1[:, ev, ts(c, 128)], ident)
                dst = att1T[:, :, e * 256 + vt * 128: e * 256 + (vt + 1) * 128]
                srcv = ps_tr[:].rearrange("p (c x) -> p c x", c=4)
                if act_evac:
                    nc.scalar.copy(dst, srcv)
                else:
                    evac(dst, srcv)
            if fast_ln1:
                att1t_ev(0, True)
                att1t_ev(1, True)
            if l + 1 < L:
                emit_kv(l + 1, 0)
            if not fast_ln1:
                rs1, nb1 = ln_batch(mv1)
            for ev in range(4):
                if fast_ln1:
                    if ev < 2:
                        continue
                    nc.vector.tensor_scalar_sub(
                        att1[:, ev, :], att_res[:, ev, :], mv1[:, ev, 0:1])
                else:
                    ln_apply(att1[:, ev, :], att_res[:, ev, :], rs1, nb1,
                             ev, l, 1)
                att1t_ev(ev, False)

            # ---- FF (biases + relu folded into ACT evacs) ----
            ff1T = ftpool.tile([128, 4, NV2], BF16, tag="ffT")
            for mt in range(4):
                ps = ps_b.tile([128, 512], F32, tag="psb")
                mm(ps[:], [(ffw1_t[:, c, ts(mt, 128)], att1T[:, c, :])
                           for c in range(4)])
                nc.scalar.activation(ff1T[:, mt, :], ps[:], AF.Relu,
                                     bias=ffb_t[:, l * 12 + mt: l * 12 + mt + 1])
            ff2T = ftpool.tile([128, 4, NV2], BF16, tag="ffT")
            for mt in range(4):
                ps = ps_b.tile([128, 512], F32, tag="psb")
                mm(ps[:], [(ffw2_t[:, c, ts(mt, 128)], ff1T[:, c, :])
                           for c in range(4)])
                nc.scalar.activation(ff2T[:, mt, :], ps[:], AF.Relu,
                                     bias=ffb_t[:, l * 12 + 4 + mt: l * 12 + 5 + mt])
            f3T = ftpool.tile([128, 4, NV2], BF16, tag="ffT")
            for c in range(4):
                ps = ps_b.tile([128, 512], F32, tag="psb")
                mm(ps[:], [(ffw3_t[:, k, ts(c, 128)], ff2T[:, k, :])
                           for k in range(4)])
                nc.scalar.activation(f3T[:, c, :], ps[:], AF.Identity,
                                     bias=ffb_t[:, l * 12 + 8 + c: l * 12 + 9 + c])
            att2_res = rpool.tile([128, 4, HA], F32, tag="attres")
            mv2 = spool.tile([128, 4, 2], F32, tag="mv")
            for ev in range(4):
                e, vt = divmod(ev, 2)
                ps_tr = ps_b.tile([128, 512], BF16, tag="psb")
                for c in range(4):
                    nc.tensor.transpose(
                        ps_tr[:, ts(c, 128)],
                        f3T[:, c, e * 256 + vt * 128: e * 256 + (vt + 1) * 128],
                        ident)
                nc.vector.tensor_add(att2_res[:, ev, :],
                                     ps_tr[:], att1[:, ev, :])
                ln_stats(mv2, att2_res[:, ev, :], ev)

            # ---- LN2 + att2T, with next layer's KV filling the bubble ----
            rs2, nb2 = ln_batch(mv2)
            defer2 = fast_head and l == L - 1
            if defer2:
                head_rs = rs2
            if l + 1 < L:
                emit_kv(l + 1, 1)
            att2 = apool.tile([128, 4, HA], BF16, tag="att")
            att2T = tpool.tile([128, 4, NV2], BF16, tag="attT")
            for ev in range(4):
                e, vt = divmod(ev, 2)
                if defer2:
                    # last layer feeds only the loss head (de_b == 0), so the
                    # 1/std scale can be applied inside the head instead:
                    # exp(scale*logits) on ACT and pick*rstd on DVE.
                    nc.vector.tensor_scalar_sub(
                        att2[:, ev, :], att2_res[:, ev, :], mv2[:, ev, 0:1])
                else:
                    ln_apply(att2[:, ev, :], att2_res[:, ev, :], rs2, nb2,
                             ev, l, 2)
                ps_tr = ps_b.tile([128, 512], BF16, tag="psb")
                for c in range(4):
                    nc.tensor.transpose(ps_tr[:, ts(c, 128)],
                                        att2[:, ev, ts(c, 128)], ident)
                evac(att2T[:, :, e * 256 + vt * 128: e * 256 + (vt + 1) * 128],
                     ps_tr[:].rearrange("p (c x) -> p c x", c=4))
            if l + 1 < L:
                emit_kv(l + 1, 2)
            att, attT = att2, att2T
            ffw1_t, ffw2_t, ffw3_t = None, None, None

        # ================== loss head ==================
        q = spool.tile([128, 4], F32, tag="q")
        se4 = spool.tile([128, 4], F32, tag="se4")
        pick4 = spool.tile([128, 4], F32, tag="pick4")
        for ev in range(4):
            e, vt = divmod(ev, 2)
            ps = ps_b.tile([128, 512], F32, tag="psb")
            ch = [(attT[:, c, e * 256 + vt * 128: e * 256 + (vt + 1) * 128],
                   dew_t[:, c, :]) for c in range(4)]
            if not fast_head:
                ch.append((ones_row, deb_t))
            mm(ps[:, 0:R], ch)
            scr = spool.tile([128, R], F32, tag="scr")
            if fast_head:
                nc.scalar.activation(scr[:], ps[:, 0:R], AF.Exp,
                                     scale=head_rs[:, ev:ev + 1],
                                     accum_out=se4[:, ev:ev + 1])
            else:
                nc.scalar.activation(scr[:], ps[:, 0:R], AF.Exp,
                                     accum_out=se4[:, ev:ev + 1])
            nc.vector.tensor_mul(scr[:], ps[:, 0:R], onehot_t[:, ev, :])
            nc.vector.tensor_reduce(pick4[:, ev:ev + 1], scr[:],
                                    mybir.AxisListType.X, ALU.add)
        if fast_head:
            nc.vector.tensor_mul(pick4[:], pick4[:], head_rs[:])
        lse4 = spool.tile([128, 4], F32, tag="lse4")
        nc.scalar.activation(lse4[:], se4[:], AF.Ln)
        nc.vector.scalar_tensor_tensor(q[:], lse4[:], -1.0, pick4[:],
                                       op0=ALU.mult, op1=ALU.add)
        # zero the v=0 entry of each element's first v-chunk
        nc.vector.tensor_mul(q[:, 0:1], q[:, 0:1], wv0)
        nc.vector.tensor_mul(q[:, 2:3], q[:, 2:3], wv0)
        ps_l = ps_a.tile([128, 66], F32, tag="a")
        nc.tensor.matmul(ps_l[0:1, 0:4], ones_col, q[:, 0:4],
                         start=True, stop=True)
        tot = spool.tile([1, EPC], F32, tag="tot")
        nc.vector.tensor_reduce(
            tot[:], ps_l[0:1, 0:4].rearrange("p (e k) -> p e k", e=2),
            mybir.AxisListType.X, ALU.add)
        nc.scalar.activation(res_sb[0:1, :], tot[0:1, :], AF.Identity,
                             scale=neg1_t[0:1, 0:1], bias=fbias_t[0:1, 0:1])
        dma(out_d.ap()[0:1, :], res_sb[:])

    nc.finalize()
    return nc


def _prep_inputs(inputs):
    f32 = lambda k: np.asarray(inputs[k], np.float32)
    hist_encoded = f32("hist_encoded")
    hist_true_u = f32("hist_true_u")
    pred_encoded = f32("pred_encoded")
    pred_true_u = f32("pred_true_u")
    key_w, key_b = f32("key_w"), f32("key_b")
    val_w, val_b = f32("val_w"), f32("val_b")
    ds_w, ds_b = f32("ds_w"), f32("ds_b")
    ff_w1, ff_b1 = f32("ff_w1"), f32("ff_b1")
    ff_w2, ff_b2 = f32("ff_w2"), f32("ff_b2")
    ff_w3, ff_b3 = f32("ff_w3"), f32("ff_b3")
    de_w, de_b = f32("de_w"), f32("de_b")
    ln1_g, ln1_b = f32("ln1_g"), f32("ln1_b")
    ln2_g, ln2_b = f32("ln2_g"), f32("ln2_b")

    enc = np.concatenate([hist_encoded, pred_encoded], axis=1)  # [B, W, D]
    u = np.concatenate([hist_true_u, pred_true_u], axis=1)      # [B, W]
    encT = np.ascontiguousarray(enc.transpose(0, 2, 1))         # [B, D, W]

    # weight packs (shared across cores), bf16
    dsw = np.ascontiguousarray(
        ds_w.reshape(2, 128, HA).transpose(1, 0, 2)).astype(BF)
    dsb = np.ascontiguousarray(ds_b.reshape(4, 128).T)

    def pack_w(wt):  # [L, 256, HA] -> [L, 128, 2, HA]
        return np.ascontiguousarray(
            wt.reshape(L, 2, 128, HA).transpose(0, 2, 1, 3)).astype(BF)

    kwt = key_w.transpose(0, 2, 1, 3).reshape(L, D + 1, HA)  # [L, 257, HA]
    vwt = val_w.transpose(0, 2, 1, 3).reshape(L, D + 1, HA)
    kw = pack_w(kwt[:, 0:256])
    vw = pack_w(vwt[:, 0:256])
    kwu = np.ascontiguousarray(
        kwt[:, 256].reshape(L, 4, 128).transpose(0, 2, 1))  # [L, 128, 4] f32
    vwu = np.repeat(vwt[:, 256][:, None, :], 128, axis=1).astype(BF)  # [L,128,HA]
    kv_bias = bool(np.any(key_b) or np.any(val_b))
    kbc = np.ascontiguousarray(
        key_b.reshape(L, 4, 128).transpose(0, 2, 1)).astype(np.float32)
    vbb = np.repeat(val_b.reshape(L, HA)[:, None, :], 128, axis=1).astype(BF)

    def pack_ff(wt, n):  # [L, 512, n] -> [L, 128, 4, n]
        return np.ascontiguousarray(
            wt.reshape(L, 4, 128, n).transpose(0, 2, 1, 3)).astype(BF)

    f1 = pack_ff(ff_w1, M)
    f2 = pack_ff(ff_w2, M)
    f3 = pack_ff(ff_w3, HA)
    ffb = np.empty((128, L * 12), np.float32)
    for l in range(L):
        ffb[:, l * 12 + 0: l * 12 + 4] = ff_b1[l].reshape(4, 128).T
        ffb[:, l * 12 + 4: l * 12 + 8] = ff_b2[l].reshape(4, 128).T
        ffb[:, l * 12 + 8: l * 12 + 12] = ff_b3[l].reshape(4, 128).T

    dew = np.ascontiguousarray(
        de_w.reshape(4, 128, R).transpose(1, 0, 2)).astype(BF)
    deb = de_b.reshape(1, R).astype(BF)

    rho = np.arange(128)[:, None]
    vv = np.arange(128)[None, :]
    maskm = (vv > rho).astype(BF)  # 0 where v <= w' (masked)

    ident = np.eye(128, dtype=np.float32).astype(BF)
    wv0 = np.ones((128, 1), np.float32)
    wv0[0, 0] = 0.0

    tgt = np.clip(np.floor(pred_true_u * R).astype(np.int64), 0, R - 1)  # [B, NV]

    ln_affine = bool(np.any(ln1_g != 1.0) or np.any(ln1_b) or
                     np.any(ln2_g != 1.0) or np.any(ln2_b))
    ffb_zero = not (np.any(ff_b1) or np.any(ff_b2) or np.any(ff_b3))
    fast_ln1 = (not ln_affine) and ffb_zero
    fast_head = (not ln_affine) and not np.any(de_b)
    flags = (ln_affine, kv_bias, fast_ln1, fast_head)
    lnp = np.stack([ln1_g, ln1_b, ln2_g, ln2_b], axis=1).reshape(1, -1)

    # ---- pack blobs (one DMA each device-side) ----
    CBN = 3328
    CFN = 610 if kv_bias else 594
    WBN = 9216 if kv_bias else 8704
    cb = np.zeros((128, CBN), BF)
    cb[:, 0:128] = ident
    cb[:, 128:256] = maskm
    cb[:, 256:1280] = dsw.reshape(128, 1024)
    cb[:, 1280:1792] = dew.reshape(128, 512)
    # ubc filled per core below
    cf = np.zeros((128, CFN), np.float32)
    cf[:, 0:4] = dsb
    # ucol per core below
    cf[:, 16:64] = ffb
    # onehot per core below
    cf[:, 576] = 1.0
    cf[:, 577:578] = wv0
    cf[:, 578:594] = kwu.transpose(1, 0, 2).reshape(128, 16)
    if kv_bias:
        cf[:, 594:610] = kbc.transpose(1, 0, 2).reshape(128, 16)
    wb = np.zeros((L, 128, WBN), BF)
    wb[:, :, 0:1024] = kw.reshape(L, 128, 1024)
    wb[:, :, 1024:2048] = vw.reshape(L, 128, 1024)
    wb[:, :, 2048:2560] = vwu
    wb[:, :, 2560:4608] = f1.reshape(L, 128, 2048)
    wb[:, :, 4608:6656] = f2.reshape(L, 128, 2048)
    wb[:, :, 6656:8704] = f3.reshape(L, 128, 2048)
    if kv_bias:
        wb[:, :, 8704:9216] = vbb
    srow = np.zeros((1, 256), BF)
    srow[0, 0:128] = 1.0
    srow[0, 128:256] = deb[0]
    shared = {
        "cf": cf, "wb": wb, "smallrow": srow,
    }
    if ln_affine:
        shared["lnp"] = lnp

    in_maps = []
    for c in range(NCORES):
        m = dict(shared)
        e0, e1 = 2 * c, 2 * c + 1
        ki = np.empty((258, W2), np.float32)
        ki[0:256, 0:W] = encT[e0]
        ki[0:256, W:] = encT[e1]
        ki[256, 0:W] = u[e0]
        ki[256, W:] = u[e1]
        ki[257, :] = 1.0
        kib = ki.astype(BF)
        m["ki0"] = np.ascontiguousarray(kib[0:128])
        m["ki1"] = np.ascontiguousarray(kib[128:256])
        ucat = ki[256]  # [W2] f32
        cbm = cb.copy()
        cbm[:, 1792:3328] = np.repeat(ucat[None, :], 128, axis=0).astype(BF)
        m["cb"] = cbm
        cfm = cf.copy()
        cfm[:, 4:16] = ucat.reshape(12, 128).T
        onehot = np.zeros((128, 4, R), np.float32)
        for ev in range(4):
            e, vt = divmod(ev, 2)
            idx = tgt[2 * c + e, vt * 128:(vt + 1) * 128]
            onehot[np.arange(128), ev, idx] = 1.0
        onehot[0, 0, :] = 0.0  # v=0 excluded (e0)
        onehot[0, 2, :] = 0.0  # v=0 excluded (e1)
        cfm[:, 64:576] = onehot.reshape(128, 512)
        m["cf"] = cfm
        in_maps.append(m)
    return in_maps, flags


def _get_nc(flags):
    if flags not in _BUILD_CACHE:
        _BUILD_CACHE[flags] = _build(*flags)
    return _BUILD_CACHE[flags]


def _run(inputs, trace=False):
    from concourse.bass_utils import run_bass_kernel_spmd
    in_maps, flags = _prep_inputs(inputs)
    nc = _get_nc(flags)
    res = run_bass_kernel_spmd(nc, in_maps, list(range(NCORES)), trace=trace)
    out = np.concatenate([res.results[c]["out"].reshape(EPC)
                          for c in range(NCORES)])
    return out.astype(np.float32), res


def kernel(**inputs) -> np.ndarray:
    out, _ = _run(inputs, trace=False)
    return out


# revision 24
# speedup vs baseline: 1.0471x; 1.0471x over previous
"""AttentionalCopula Trainium2 kernel, v2.

Data-parallel over batch: 8 NeuronCores, 2 batch elements per core,
both elements merged into one instruction stream (weights shared, free
dim N=512).  All matmuls in bf16 (fp32 PSUM accumulate); vector math in
fp32 where it matters (LN stats, softmax denominators, loss head).
Biases folded into ACT-engine PSUM evacuations (per-partition bias),
u-row handled as a K=2 matmul pass.  Fully-masked score block skipped.
Single ACT table set (exp/ln/identity/relu) to avoid table reloads.

Self-contained: hardcodes shapes from the problem spec.
"""
import math
import sys

import numpy as np
import ml_dtypes

sys.path.insert(0, "/opt/trn_rl_repo")

import concourse.bass as bass  # noqa: E402
import concourse.bacc as bacc  # noqa: E402
import concourse.tile as tile  # noqa: E402
import concourse.mybir as mybir  # noqa: E402
from contextlib import ExitStack  # noqa: E402

F32 = mybir.dt.float32
BF16 = mybir.dt.bfloat16
AF = mybir.ActivationFunctionType
ALU = mybir.AluOpType

B, D, NH, NS, NT = 16, 256, 512, 8, 32
NV = NS * NT          # 256
L, H, A = 4, 8, 64
HA = H * A            # 512
M = 512
R = 128
W = NH + NV           # 768
EPS = 1e-5
SCALE = A ** -0.5
NCORES = 8
EPC = B // NCORES     # 2 elems per core
W2 = 2 * W            # 1536 (both elems, e-major)
NV2 = 2 * NV          # 512

BF = ml_dtypes.bfloat16

_BUILD_CACHE = {}


def ts(i, n):
    return slice(i * n, (i + 1) * n)


def _build(ln_affine, kv_bias, fast_ln1, fast_head):
    nc = bacc.Bacc(None, target_bir_lowering=False)

    def P(name, shape, out=False, dt=BF16):
        return nc.declare_dram_parameter(name, shape, dt, isOutput=out)

    # single-DMA blobs: SP-queue DMA issue costs ~1us each, so pack
    # aggressively.  bf16 const blob layout (cols):
    #   ident 0:128 | maskm 128:256 | dsw 256:1280 | dew 1280:1792
    #   | ubc 1792:3328
    # f32 const blob layout (cols):
    #   dsb 0:4 | ucol 4:16 | ffb 16:64 | onehot 64:576 | onescol 576
    #   | wv0 577 | kwu 578:594 | kbc 594:610 (kv_bias only)
    # per-layer weight blob (bf16):
    #   kw 0:1024 | vw 1024:2048 | vwu 2048:2560 | f1 2560:4608
    #   | f2 4608:6656 | f3 6656:8704 | vbb 8704:9216 (kv_bias only)
    CBN = 3328
    CFN = 610 if kv_bias else 594
    WBN = 9216 if kv_bias else 8704
    cb_d = P("cb", (128, CBN))
    cf_d = P("cf", (128, CFN), dt=F32)
    wb_d = P("wb", (L, 128, WBN))
    ki0_d = P("ki0", (128, W2))
    ki1_d = P("ki1", (128, W2))
    sr_d = P("smallrow", (1, 256))
    if ln_affine:
        lnp_d = P("lnp", (1, L * 4 * HA), dt=F32)
    out_d = P("out", (1, EPC), out=True, dt=F32)

    with tile.TileContext(nc) as tc, ExitStack() as ctx:
        const = ctx.enter_context(tc.tile_pool(name="const", bufs=1))
        kpool = ctx.enter_context(tc.tile_pool(name="kvw", bufs=2))
        fpool = ctx.enter_context(tc.tile_pool(name="ffw", bufs=2))
        kvpool = ctx.enter_context(tc.tile_pool(name="keys", bufs=2))
        epool = ctx.enter_context(tc.tile_pool(name="exp", bufs=3))
        apool = ctx.enter_context(tc.tile_pool(name="att", bufs=3))
        rpool = ctx.enter_context(tc.tile_pool(name="attres", bufs=2))
        tpool = ctx.enter_context(tc.tile_pool(name="attT", bufs=3))
        ftpool = ctx.enter_context(tc.tile_pool(name="ffT", bufs=3))
        spool = ctx.enter_context(tc.tile_pool(name="small", bufs=4))
        ps_b = ctx.enter_context(tc.tile_pool(name="ps_b", bufs=2, space="PSUM"))
        ps_sc = ctx.enter_context(tc.tile_pool(name="ps_sc", bufs=2, space="PSUM"))
        ps_a = ctx.enter_context(tc.tile_pool(name="ps_a", bufs=2, space="PSUM"))

        dma = nc.sync.dma_start

        # ---- constants / inputs (few big DMAs) ----
        cb = const.tile([128, CBN], BF16, tag="cb")
        dma(cb[:], cb_d.ap())
        ki0 = const.tile([128, W2], BF16, tag="ki0")
        dma(ki0[:], ki0_d.ap())
        ki1 = const.tile([128, W2], BF16, tag="ki1")
        dma(ki1[:], ki1_d.ap())
        cf = const.tile([128, CFN], F32, tag="cf")
        dma(cf[:], cf_d.ap())
        srow = const.tile([1, 256], BF16, tag="srow")
        dma(srow[:], sr_d.ap())
        ident = cb[:, 0:128]
        maskm = cb[:, 128:256]
        dsw_t = cb[:, 256:1280].rearrange("p (a n) -> p a n", a=2)
        dew_t = cb[:, 1280:1792].rearrange("p (a n) -> p a n", a=4)
        u_bcast = cb[:, 1792:3328]
        dsb_t = cf[:, 0:4]
        u_col = cf[:, 4:16]
        ffb_t = cf[:, 16:64]
        onehot_t = cf[:, 64:576].rearrange("p (a n) -> p a n", a=4)
        ones_col = cf[:, 576:577]
        wv0 = cf[:, 577:578]
        ones_row = srow[0:1, 0:128]
        deb_t = srow[0:1, 128:256]
        if ln_affine:
            lnp_t = const.tile([1, 16, HA], F32, tag="lnp")
            dma(lnp_t[:], lnp_d.ap().rearrange("p (a n) -> p a n", a=16))
        eps_t = const.tile([128, 1], F32, tag="eps")
        nc.gpsimd.memset(eps_t[:], EPS)
        sc8_t = const.tile([128, 1], F32, tag="sc8")
        nc.gpsimd.memset(sc8_t[:], SCALE)
        neg1_t = const.tile([1, 1], F32, tag="neg1")
        nc.gpsimd.memset(neg1_t[:], -1.0)
        fbias_t = const.tile([1, 1], F32, tag="fbias")
        nc.gpsimd.memset(fbias_t[:], -(NV - 1) * math.log(R))
        res_sb = const.tile([1, EPC], F32, tag="res")

        evac_ctr = [0]

        def evac(out_ap, in_ap):
            # PSUM->SBUF copies alternating DVE / ACT
            if evac_ctr[0] % 2 == 0:
                nc.vector.tensor_copy(out_ap, in_ap)
            else:
                nc.scalar.copy(out_ap, in_ap)
            evac_ctr[0] += 1

        def mm(ps_ap, chunks):
            n = len(chunks)
            for i, (lh, rh) in enumerate(chunks):
                nc.tensor.matmul(ps_ap, lh, rh,
                                 start=(i == 0), stop=(i == n - 1))

        def ln_stats(mvall, in_ap, ev):
            """bn stats for one [128, HA] chunk -> mvall[:, ev, 0:2]."""
            st6 = spool.tile([128, 6], F32, tag="st6")
            nc.vector.bn_stats(st6[:], in_ap)
            nc.vector.bn_aggr(mvall[:, ev, :], st6[:])

        def ln_batch(mvall):
            """rstd/-mean*rstd for all 4 chunks in two ACT calls.
            rstd via exp(-0.5*ln(var+eps)) keeps Ln/Exp adjacent so only
            two table loads per LN phase."""
            lv = spool.tile([128, 4], F32, tag="lv")
            nc.scalar.activation(lv[:], mvall[:, :, 1], AF.Ln,
                                 bias=eps_t[:, 0:1])
            rs = spool.tile([128, 4], F32, tag="rs")
            nc.scalar.activation(rs[:], lv[:], AF.Exp, scale=-0.5)
            nb = spool.tile([128, 4], F32, tag="nb")
            nc.vector.scalar_tensor_tensor(nb[:], mvall[:, :, 0], -1.0, rs[:],
                                           op0=ALU.mult, op1=ALU.mult)
            return rs, nb

        def ln_apply(out_ap, in_ap, rs, nb, ev, l, which):
            """normalize on DVE: out = in*rstd - mean*rstd."""
            if not ln_affine:
                nc.vector.tensor_scalar(out_ap, in_ap,
                                        rs[:, ev:ev + 1], nb[:, ev:ev + 1],
                                        op0=ALU.mult, op1=ALU.add)
            else:
                t0 = spool.tile([128, HA], F32, tag="lnt0", bufs=1)
                nc.vector.tensor_scalar(t0[:], in_ap,
                                        rs[:, ev:ev + 1], nb[:, ev:ev + 1],
                                        op0=ALU.mult, op1=ALU.add)
                gb = spool.tile([128, HA], F32, tag="lngb", bufs=1)
                gi = l * 4 + (0 if which == 1 else 2)
                nc.gpsimd.partition_broadcast(gb[:], lnp_t[0:1, gi, :])
                nc.vector.tensor_mul(t0[:], t0[:], gb[:])
                nc.gpsimd.partition_broadcast(gb[:], lnp_t[0:1, gi + 1, :])
                nc.vector.tensor_add(out_ap, t0[:], gb[:])

        # vals denominator column (col 64 = 1.0, written once; col 65 is
        # never read).  GPSIMD memsets instead of a slow strided DMA.
        vals_a = kvpool.tile([128, 12, 8, 66], BF16, tag="vals")
        vals_b = kvpool.tile([128, 12, 8, 66], BF16, tag="vals")
        for vt_ in (vals_a, vals_b):
            for wt in range(12):
                nc.gpsimd.memset(vt_[:, wt, :, 64:65], 1.0)
        vals_bufs = [vals_a, vals_b]

        # ================== initial att / attT ==================
        # attT[ha, (e,v)] = ds_w.T @ pred_encoded.T + ds_b (per-partition bias)
        attT = tpool.tile([128, 4, NV2], BF16, tag="attT")
        for t in range(4):
            ps = ps_b.tile([128, 512], F32, tag="psb")
            for e in range(EPC):
                nc.tensor.matmul(ps[:, ts(e, 256)],
                                 dsw_t[:, 0, ts(t, 128)],
                                 ki0[:, e * W + NH: (e + 1) * W],
                                 start=True, stop=False)
                nc.tensor.matmul(ps[:, ts(e, 256)],
                                 dsw_t[:, 1, ts(t, 128)],
                                 ki1[:, e * W + NH: (e + 1) * W],
                                 start=False, stop=True)
            nc.scalar.activation(attT[:, t, :], ps[:], AF.Identity,
                                 bias=dsb_t[:, t: t + 1])
        # att natural [v, (e,vt,ha)] via PE transposes
        att = apool.tile([128, 4, HA], BF16, tag="att")
        for ev in range(4):
            e, vt = divmod(ev, 2)
            ps = ps_b.tile([128, 512], BF16, tag="psb")
            for t in range(4):
                nc.tensor.transpose(ps[:, ts(t, 128)],
                                    attT[:, t, e * 256 + vt * 128:
                                         e * 256 + (vt + 1) * 128],
                                    ident)
            evac(att[:, ev, :], ps[:])

        # ================== layers ==================
        def load_weights(l):
            wt = kpool.tile([128, WBN], BF16, tag="wb")
            dma(wt[:], wb_d.ap()[l])
            kw_t = wt[:, 0:1024].rearrange("p (a n) -> p a n", a=2)
            vw_t = wt[:, 1024:2048].rearrange("p (a n) -> p a n", a=2)
            vwu_t = wt[:, 2048:2560]
            ffw1_t = wt[:, 2560:4608].rearrange("p (a n) -> p a n", a=4)
            ffw2_t = wt[:, 4608:6656].rearrange("p (a n) -> p a n", a=4)
            ffw3_t = wt[:, 6656:8704].rearrange("p (a n) -> p a n", a=4)
            kwu_t = cf[:, 578 + l * 4: 578 + (l + 1) * 4]
            kbc_t = cf[:, 594 + l * 4: 594 + (l + 1) * 4] if kv_bias else None
            vbb_t = wt[:, 8704:9216] if kv_bias else None
            return (kw_t, vw_t, kwu_t, vwu_t, kbc_t, vbb_t,
                    ffw1_t, ffw2_t, ffw3_t)

        wts = {0: load_weights(0)}
        kv_tiles = {}

        def emit_kv(l, part):
            """KV compute for layer l, split so part 0 can fill the LN2
            bubble of layer l-1 and part 1 runs after att2T transposes."""
            kw_t, vw_t, kwu_t, vwu_t, kbc_t, vbb_t = wts[l][:6]
            if part == 0:
                keysT = kvpool.tile([128, 4, W2], BF16, tag="keys")
                kv_tiles[l] = (keysT, vals_bufs[l % 2])
                rng_v, rng_k = range(0, 6), []
            elif part == 1:
                rng_v, rng_k = range(6, 12), []
            else:
                rng_v = range(0, 0)
                rng_k = [(t, ch) for t in range(4) for ch in range(3)]
            keysT, vals = kv_tiles[l]
            for wt in rng_v:
                ps = ps_b.tile([128, 512], F32, tag="psb")
                mm(ps[:], [(ki0[:, ts(wt, 128)], vw_t[:, 0, :]),
                           (ki1[:, ts(wt, 128)], vw_t[:, 1, :])])
                # u-term folded into the evac: vals += u[w] * vw_u[ha]
                nc.vector.scalar_tensor_tensor(
                    vals[:, wt, :, 0:64],
                    vwu_t.rearrange("p (h a) -> p h a", h=8),
                    u_col[:, wt:wt + 1],
                    ps[:].rearrange("p (h a) -> p h a", h=8),
                    op0=ALU.mult, op1=ALU.add)
                if kv_bias:
                    nc.vector.tensor_add(
                        vals[:, wt, :, 0:64], vals[:, wt, :, 0:64],
                        vbb_t.rearrange("p (h a) -> p h a", h=8))
            for t, ch in rng_k:
                ps = ps_b.tile([128, 512], F32, tag="psb")
                mm(ps[:], [(kw_t[:, 0, ts(t, 128)], ki0[:, ts(ch, 512)]),
                           (kw_t[:, 1, ts(t, 128)], ki1[:, ts(ch, 512)])])
                # u-term folded into the evac: keys += kw_u[ha] * u[w]
                nc.vector.scalar_tensor_tensor(
                    keysT[:, t, ts(ch, 512)],
                    u_bcast[:, ts(ch, 512)], kwu_t[:, t:t + 1], ps[:],
                    op0=ALU.mult, op1=ALU.add)
                if kv_bias:
                    nc.vector.tensor_scalar_add(
                        keysT[:, t, ts(ch, 512)], keysT[:, t, ts(ch, 512)],
                        kbc_t[:, t:t + 1])

        emit_kv(0, 0)
        emit_kv(0, 1)
        emit_kv(0, 2)

        for l in range(L):
            if l + 1 < L:
                wts[l + 1] = load_weights(l + 1)
            keysT, vals = kv_tiles[l]
            ffw1_t, ffw2_t, ffw3_t = wts[l][6], wts[l][7], wts[l][8]

            # ---- attention ----
            att_res = rpool.tile([128, 4, HA], F32, tag="attres")
            mv1 = spool.tile([128, 4, 2], F32, tag="mv")
            att1 = apool.tile([128, 4, HA], BF16, tag="att")
            att1T = tpool.tile([128, 4, NV2], BF16, tag="attT")
            for e in range(EPC):
                for h in range(H):
                    t, base = h // 2, (h % 2) * 64
                    kslc = keysT[base:base + 64, t, :]
                    aslc = attT[base:base + 64, t, :]
                    # scores S^T[w, v] in two psum tiles of 3 w-chunks each
                    psA = ps_sc.tile([128, 768], F32, tag="sc")
                    for wt in range(3):
                        nc.tensor.matmul(
                            psA[:, ts(wt, 256)],
                            kslc[:, e * W + wt * 128: e * W + (wt + 1) * 128],
                            aslc[:, ts(e, 256)], start=True, stop=True)
                    psB = ps_sc.tile([128, 768], F32, tag="sc")
                    for wt in range(3, 5):
                        nc.tensor.matmul(
                            psB[:, ts(wt - 3, 256)],
                            kslc[:, e * W + wt * 128: e * W + (wt + 1) * 128],
                            aslc[:, ts(e, 256)], start=True, stop=True)
                    # wt=5: v-chunk 0 fully masked -> compute v-chunk 1 only
                    nc.tensor.matmul(
                        psB[:, 640:768],
                        kslc[:, e * W + 640: e * W + 768],
                        aslc[:, e * 256 + 128: e * 256 + 256],
                        start=True, stop=True)
                    expT = epool.tile([128, 1536], BF16, tag="exp")
                    nc.scalar.activation(expT[:, 0:768], psA[:], AF.Exp,
                                         scale=sc8_t[:, 0:1])
                    nc.scalar.activation(expT[:, 768:1536], psB[:], AF.Exp,
                                         scale=sc8_t[:, 0:1])
                    # masking on GPSIMD (SBUF-only engine, otherwise idle)
                    nc.gpsimd.memset(expT[:, 1280:1408], 0.0)
                    nc.gpsimd.tensor_mul(expT[:, 1024:1152],
                                         expT[:, 1024:1152], maskm[:])
                    nc.gpsimd.tensor_mul(expT[:, 1408:1536],
                                         expT[:, 1408:1536], maskm[:])
                    # attention-out directly in [v, a] layout:
                    # lhsT = expT v-slice, rhs = vals (with ones col 64
                    # giving the softmax denominator at out col 64)
                    rec = spool.tile([128, 2], F32, tag="rec")
                    for vc in range(2):
                        ps_at = ps_a.tile([128, 66], F32, tag="a")
                        wts_ao = range(5) if vc == 0 else range(6)
                        last = wts_ao[-1]
                        for wt in wts_ao:
                            nc.tensor.matmul(
                                ps_at[:],
                                expT[:, wt * 256 + vc * 128:
                                     wt * 256 + (vc + 1) * 128],
                                vals[:, e * 6 + wt, h, :],
                                start=(wt == 0), stop=(wt == last))
                        nc.vector.reciprocal(rec[:, vc:vc + 1],
                                             ps_at[:, 64:65])
                        nc.vector.scalar_tensor_tensor(
                            att_res[:, e * 2 + vc, ts(h, 64)],
                            ps_at[:, 0:64], rec[:, vc:vc + 1],
                            att[:, e * 2 + vc, ts(h, 64)],
                            op0=ALU.mult, op1=ALU.add)
                # LN1 stats for this element's two halves, overlapped with
                # the other element's attention
                ln_stats(mv1, att_res[:, e * 2, :], e * 2)
                ln_stats(mv1, att_res[:, e * 2 + 1, :], e * 2 + 1)
                if fast_ln1 and e == 0:
                    # e0's mean-subtract runs during e1's attention
                    for ev in (0, 1):
                        nc.vector.tensor_scalar_sub(
                            att1[:, ev, :], att_res[:, ev, :], mv1[:, ev, 0:1])

            # ---- LN1 + att1T ----
            # ev 0/1 were mean-subtracted during e1's attention, so their
            # transposes go FIRST (psum ring + ACT evacs, nothing queued
            # behind the vals evacs on DVE); then next layer's vals groups
            # keep the PE busy while ev 2/3 resolve.
            def att1t_ev(ev, act_evac):
                e, vt = divmod(ev, 2)
                ps_tr = ps_b.tile([128, 512], BF16, tag="psb")
                for c in range(4):
                    nc.tensor.transpose(ps_tr[:, ts(c, 128)],
                                        att1[:, ev, ts(c, 128)], ident)
                dst = att1T[:, :, e * 256 + vt * 128: e * 256 + (vt + 1) * 128]
                srcv = ps_tr[:].rearrange("p (c x) -> p c x", c=4)
                if act_evac:
                    nc.scalar.copy(dst, srcv)
                else:
                    evac(dst, srcv)
            if l + 1 < L:
                emit_kv(l + 1, 0)
            if not fast_ln1:
                rs1, nb1 = ln_batch(mv1)
            for ev in range(4):
                if fast_ln1:
                    if ev >= 2:
                        nc.vector.tensor_scalar_sub(
                            att1[:, ev, :], att_res[:, ev, :], mv1[:, ev, 0:1])
                else:
                    ln_apply(att1[:, ev, :], att_res[:, ev, :], rs1, nb1,
                             ev, l, 1)
                att1t_ev(ev, False)

            # ---- FF (biases + relu folded into ACT evacs) ----
            ff1T = ftpool.tile([128, 4, NV2], BF16, tag="ffT")
            for mt in range(4):
                ps = ps_b.tile([128, 512], F32, tag="psb")
                mm(ps[:], [(ffw1_t[:, c, ts(mt, 128)], att1T[:, c, :])
                           for c in range(4)])
                nc.scalar.activation(ff1T[:, mt, :], ps[:], AF.Relu,
                                     bias=ffb_t[:, l * 12 + mt: l * 12 + mt + 1])
            ff2T = ftpool.tile([128, 4, NV2], BF16, tag="ffT")
            for mt in range(4):
                ps = ps_b.tile([128, 512], F32, tag="psb")
                mm(ps[:], [(ffw2_t[:, c, ts(mt, 128)], ff1T[:, c, :])
                           for c in range(4)])
                nc.scalar.activation(ff2T[:, mt, :], ps[:], AF.Relu,
                                     bias=ffb_t[:, l * 12 + 4 + mt: l * 12 + 5 + mt])
            f3T = ftpool.tile([128, 4, NV2], BF16, tag="ffT")
            for c in range(4):
                ps = ps_b.tile([128, 512], F32, tag="psb")
                mm(ps[:], [(ffw3_t[:, k, ts(c, 128)], ff2T[:, k, :])
                           for k in range(4)])
                nc.scalar.activation(f3T[:, c, :], ps[:], AF.Identity,
                                     bias=ffb_t[:, l * 12 + 8 + c: l * 12 + 9 + c])
            att2_res = rpool.tile([128, 4, HA], F32, tag="attres")
            mv2 = spool.tile([128, 4, 2], F32, tag="mv")
            for ev in range(4):
                e, vt = divmod(ev, 2)
                ps_tr = ps_b.tile([128, 512], BF16, tag="psb")
                for c in range(4):
                    nc.tensor.transpose(
                        ps_tr[:, ts(c, 128)],
                        f3T[:, c, e * 256 + vt * 128: e * 256 + (vt + 1) * 128],
                        ident)
                nc.vector.tensor_add(att2_res[:, ev, :],
                                     ps_tr[:], att1[:, ev, :])
                ln_stats(mv2, att2_res[:, ev, :], ev)

            # ---- LN2 + att2T, with next layer's KV filling the bubble ----
            rs2, nb2 = ln_batch(mv2)
            defer2 = fast_head and l == L - 1
            if defer2:
                head_rs = rs2
            if l + 1 < L:
                emit_kv(l + 1, 1)
            att2 = apool.tile([128, 4, HA], BF16, tag="att")
            att2T = tpool.tile([128, 4, NV2], BF16, tag="attT")
            for ev in range(4):
                e, vt = divmod(ev, 2)
                if defer2:
                    # last layer feeds only the loss head (de_b == 0), so the
                    # 1/std scale can be applied inside the head instead:
                    # exp(scale*logits) on ACT and pick*rstd on DVE.
                    nc.vector.tensor_scalar_sub(
                        att2[:, ev, :], att2_res[:, ev, :], mv2[:, ev, 0:1])
                else:
                    ln_apply(att2[:, ev, :], att2_res[:, ev, :], rs2, nb2,
                             ev, l, 2)
                ps_tr = ps_b.tile([128, 512], BF16, tag="psb")
                for c in range(4):
                    nc.tensor.transpose(ps_tr[:, ts(c, 128)],
                                        att2[:, ev, ts(c, 128)], ident)
                evac(att2T[:, :, e * 256 + vt * 128: e * 256 + (vt + 1) * 128],
                     ps_tr[:].rearrange("p (c x) -> p c x", c=4))
            if l + 1 < L:
                emit_kv(l + 1, 2)
            att, attT = att2, att2T
            ffw1_t, ffw2_t, ffw3_t = None, None, None

        # ================== loss head ==================
        q = spool.tile([128, 4], F32, tag="q")
        se4 = spool.tile([128, 4], F32, tag="se4")
        pick4 = spool.tile([128, 4], F32, tag="pick4")
        for ev in range(4):
            e, vt = divmod(ev, 2)
            ps = ps_b.tile([128, 512], F32, tag="psb")
            ch = [(attT[:, c, e * 256 + vt * 128: e * 256 + (vt + 1) * 128],
                   dew_t[:, c, :]) for c in range(4)]
            if not fast_head:
                ch.append((ones_row, deb_t))
            mm(ps[:, 0:R], ch)
            scr = spool.tile([128, R], F32, tag="scr")
            if fast_head:
                nc.scalar.activation(scr[:], ps[:, 0:R], AF.Exp,
                                     scale=head_rs[:, ev:ev + 1],
                                     accum_out=se4[:, ev:ev + 1])
            else:
                nc.scalar.activation(scr[:], ps[:, 0:R], AF.Exp,
                                     accum_out=se4[:, ev:ev + 1])
            nc.vector.tensor_mul(scr[:], ps[:, 0:R], onehot_t[:, ev, :])
            nc.vector.tensor_reduce(pick4[:, ev:ev + 1], scr[:],
                                    mybir.AxisListType.X, ALU.add)
        if fast_head:
            nc.vector.tensor_mul(pick4[:], pick4[:], head_rs[:])
        lse4 = spool.tile([128, 4], F32, tag="lse4")
        nc.scalar.activation(lse4[:], se4[:], AF.Ln)
        nc.vector.scalar_tensor_tensor(q[:], lse4[:], -1.0, pick4[:],
                                       op0=ALU.mult, op1=ALU.add)
        # zero the v=0 entry of each element's first v-chunk
        nc.vector.tensor_mul(q[:, 0:1], q[:, 0:1], wv0)
        nc.vector.tensor_mul(q[:, 2:3], q[:, 2:3], wv0)
        ps_l = ps_a.tile([128, 66], F32, tag="a")
        nc.tensor.matmul(ps_l[0:1, 0:4], ones_col, q[:, 0:4],
                         start=True, stop=True)
        tot = spool.tile([1, EPC], F32, tag="tot")
        nc.vector.tensor_reduce(
            tot[:], ps_l[0:1, 0:4].rearrange("p (e k) -> p e k", e=2),
            mybir.AxisListType.X, ALU.add)
        nc.scalar.activation(res_sb[0:1, :], tot[0:1, :], AF.Identity,
                             scale=neg1_t[0:1, 0:1], bias=fbias_t[0:1, 0:1])
        dma(out_d.ap()[0:1, :], res_sb[:])

    nc.finalize()
    return nc


def _prep_inputs(inputs):
    f32 = lambda k: np.asarray(inputs[k], np.float32)
    hist_encoded = f32("hist_encoded")
    hist_true_u = f32("hist_true_u")
    pred_encoded = f32("pred_encoded")
    pred_true_u = f32("pred_true_u")
    key_w, key_b = f32("key_w"), f32("key_b")
    val_w, val_b = f32("val_w"), f32("val_b")
    ds_w, ds_b = f32("ds_w"), f32("ds_b")
    ff_w1, ff_b1 = f32("ff_w1"), f32("ff_b1")
    ff_w2, ff_b2 = f32("ff_w2"), f32("ff_b2")
    ff_w3, ff_b3 = f32("ff_w3"), f32("ff_b3")
    de_w, de_b = f32("de_w"), f32("de_b")
    ln1_g, ln1_b = f32("ln1_g"), f32("ln1_b")
    ln2_g, ln2_b = f32("ln2_g"), f32("ln2_b")

    enc = np.concatenate([hist_encoded, pred_encoded], axis=1)  # [B, W, D]
    u = np.concatenate([hist_true_u, pred_true_u], axis=1)      # [B, W]
    encT = np.ascontiguousarray(enc.transpose(0, 2, 1))         # [B, D, W]

    # weight packs (shared across cores), bf16
    dsw = np.ascontiguousarray(
        ds_w.reshape(2, 128, HA).transpose(1, 0, 2)).astype(BF)
    dsb = np.ascontiguousarray(ds_b.reshape(4, 128).T)

    def pack_w(wt):  # [L, 256, HA] -> [L, 128, 2, HA]
        return np.ascontiguousarray(
            wt.reshape(L, 2, 128, HA).transpose(0, 2, 1, 3)).astype(BF)

    kwt = key_w.transpose(0, 2, 1, 3).reshape(L, D + 1, HA)  # [L, 257, HA]
    vwt = val_w.transpose(0, 2, 1, 3).reshape(L, D + 1, HA)
    kw = pack_w(kwt[:, 0:256])
    vw = pack_w(vwt[:, 0:256])
    kwu = np.ascontiguousarray(
        kwt[:, 256].reshape(L, 4, 128).transpose(0, 2, 1))  # [L, 128, 4] f32
    vwu = np.repeat(vwt[:, 256][:, None, :], 128, axis=1).astype(BF)  # [L,128,HA]
    kv_bias = bool(np.any(key_b) or np.any(val_b))
    kbc = np.ascontiguousarray(
        key_b.reshape(L, 4, 128).transpose(0, 2, 1)).astype(np.float32)
    vbb = np.repeat(val_b.reshape(L, HA)[:, None, :], 128, axis=1).astype(BF)

    def pack_ff(wt, n):  # [L, 512, n] -> [L, 128, 4, n]
        return np.ascontiguousarray(
            wt.reshape(L, 4, 128, n).transpose(0, 2, 1, 3)).astype(BF)

    f1 = pack_ff(ff_w1, M)
    f2 = pack_ff(ff_w2, M)
    f3 = pack_ff(ff_w3, HA)
    ffb = np.empty((128, L * 12), np.float32)
    for l in range(L):
        ffb[:, l * 12 + 0: l * 12 + 4] = ff_b1[l].reshape(4, 128).T
        ffb[:, l * 12 + 4: l * 12 + 8] = ff_b2[l].reshape(4, 128).T
        ffb[:, l * 12 + 8: l * 12 + 12] = ff_b3[l].reshape(4, 128).T

    dew = np.ascontiguousarray(
        de_w.reshape(4, 128, R).transpose(1, 0, 2)).astype(BF)
    deb = de_b.reshape(1, R).astype(BF)

    rho = np.arange(128)[:, None]
    vv = np.arange(128)[None, :]
    maskm = (vv > rho).astype(BF)  # 0 where v <= w' (masked)

    ident = np.eye(128, dtype=np.float32).astype(BF)
    wv0 = np.ones((128, 1), np.float32)
    wv0[0, 0] = 0.0

    tgt = np.clip(np.floor(pred_true_u * R).astype(np.int64), 0, R - 1)  # [B, NV]

    ln_affine = bool(np.any(ln1_g != 1.0) or np.any(ln1_b) or
                     np.any(ln2_g != 1.0) or np.any(ln2_b))
    ffb_zero = not (np.any(ff_b1) or np.any(ff_b2) or np.any(ff_b3))
    fast_ln1 = (not ln_affine) and ffb_zero
    fast_head = (not ln_affine) and not np.any(de_b)
    flags = (ln_affine, kv_bias, fast_ln1, fast_head)
    lnp = np.stack([ln1_g, ln1_b, ln2_g, ln2_b], axis=1).reshape(1, -1)

    # ---- pack blobs (one DMA each device-side) ----
    CBN = 3328
    CFN = 610 if kv_bias else 594
    WBN = 9216 if kv_bias else 8704
    cb = np.zeros((128, CBN), BF)
    cb[:, 0:128] = ident
    cb[:, 128:256] = maskm
    cb[:, 256:1280] = dsw.reshape(128, 1024)
    cb[:, 1280:1792] = dew.reshape(128, 512)
    # ubc filled per core below
    cf = np.zeros((128, CFN), np.float32)
    cf[:, 0:4] = dsb
    # ucol per core below
    cf[:, 16:64] = ffb
    # onehot per core below
    cf[:, 576] = 1.0
    cf[:, 577:578] = wv0
    cf[:, 578:594] = kwu.transpose(1, 0, 2).reshape(128, 16)
    if kv_bias:
        cf[:, 594:610] = kbc.transpose(1, 0, 2).reshape(128, 16)
    wb = np.zeros((L, 128, WBN), BF)
    wb[:, :, 0:1024] = kw.reshape(L, 128, 1024)
    wb[:, :, 1024:2048] = vw.reshape(L, 128, 1024)
    wb[:, :, 2048:2560] = vwu
    wb[:, :, 2560:4608] = f1.reshape(L, 128, 2048)
    wb[:, :, 4608:6656] = f2.reshape(L, 128, 2048)
    wb[:, :, 6656:8704] = f3.reshape(L, 128, 2048)
    if kv_bias:
        wb[:, :, 8704:9216] = vbb
    srow = np.zeros((1, 256), BF)
    srow[0, 0:128] = 1.0
    srow[0, 128:256] = deb[0]
    shared = {
        "cf": cf, "wb": wb, "smallrow": srow,
    }
    if ln_affine:
        shared["lnp"] = lnp

    in_maps = []
    for c in range(NCORES):
        m = dict(shared)
        e0, e1 = 2 * c, 2 * c + 1
        ki = np.empty((258, W2), np.float32)
        ki[0:256, 0:W] = encT[e0]
        ki[0:256, W:] = encT[e1]
        ki[256, 0:W] = u[e0]
        ki[256, W:] = u[e1]
        ki[257, :] = 1.0
        kib = ki.astype(BF)
        m["ki0"] = np.ascontiguousarray(kib[0:128])
        m["ki1"] = np.ascontiguousarray(kib[128:256])
        ucat = ki[256]  # [W2] f32
        cbm = cb.copy()
        cbm[:, 1792:3328] = np.repeat(ucat[None, :], 128, axis=0).astype(BF)
        m["cb"] = cbm
        cfm = cf.copy()
        cfm[:, 4:16] = ucat.reshape(12, 128).T
        onehot = np.zeros((128, 4, R), np.float32)
        for ev in range(4):
            e, vt = divmod(ev, 2)
            idx = tgt[2 * c + e, vt * 128:(vt + 1) * 128]
            onehot[np.arange(128), ev, idx] = 1.0
        onehot[0, 0, :] = 0.0  # v=0 excluded (e0)
        onehot[0, 2, :] = 0.0  # v=0 excluded (e1)
        cfm[:, 64:576] = onehot.reshape(128, 512)
        m["cf"] = cfm
        in_maps.append(m)
    return in_maps, flags


def _get_nc(flags):
    if flags not in _BUILD_CACHE:
        _BUILD_CACHE[flags] = _build(*flags)
    return _BUILD_CACHE[flags]


def _run(inputs, trace=False):
    from concourse.bass_utils import run_bass_kernel_spmd
    in_maps, flags = _prep_inputs(inputs)
    nc = _get_nc(flags)
    res = run_bass_kernel_spmd(nc, in_maps, list(range(NCORES)), trace=trace)
    out = np.concatenate([res.results[c]["out"].reshape(EPC)
                          for c in range(NCORES)])
    return out.astype(np.float32), res


def kernel(**inputs) -> np.ndarray:
    out, _ = _run(inputs, trace=False)
    return out
